# revision 3
# baseline (speedup 1.0000x reference)
"""Trainium2 Bass kernel for the EdgeModel GNN message-passing MLP.

Computation (per edge e):
    x = concat([src[e], dest[e], edge_attr[e], u[batch[e]]])   # [384]
    h = relu(x @ W1 + b1)                                      # [256]
    out[e] = h @ W2 + b2                                       # [64]

Sharding: data-parallel over the edge dimension E across 8 NeuronCores;
u and the MLP weights are replicated. No cross-device communication.

Device algorithm (per core, E_core = 65536 edges, tiles of 512 edges):
  - The TensorE contraction dim must live on partitions, so the x operand
    must be feature-major.  We keep W1/W2 as the stationary operand in
    their natural (feature-major) layout and transpose the activations:
      * src/dest/edge_attr tiles are loaded edge-major (contiguous DMA)
        and transposed on the PE (matmul-with-identity transpose).
      * u[batch] is folded into W1: the last contraction chunk is
        [W1_ea (64 rows); u @ W1_u (16 rows)] against a rhs of
        [edge_attr^T (64); one_hot(batch) (16)].  one_hot is built with a
        DMA-replicated batch row compared against an iota column.
      * Layer 1 emits h^T (hidden-major), which is exactly the layout
        layer 2 needs; only the final [64, e] output tile needs a
        transpose back to edge-major before the contiguous store.
"""

import os
import sys

for _p in ("/opt/trn_rl_repo", os.path.expanduser("~/.axon_site/_ro/trn_rl_repo")):
    if os.path.isdir(_p) and _p not in sys.path:
        sys.path.insert(0, _p)

from contextlib import ExitStack

import numpy as np

import concourse.bacc as bacc
import concourse.bass as bass
import concourse.mybir as mybir
import concourse.tile as tile
from concourse.bass_utils import run_bass_kernel_spmd
from concourse.masks import make_identity

N_CORES = 8
E_FULL = 524288
E_CORE = E_FULL // N_CORES
NODE_IN = 128
EDGE_IN = 64
GLOBAL_IN = 64
B_GLOBAL = 16
HIDDEN = 256
EDGE_OUT = 64
P = 128
TILE_E = 512
SUB = TILE_E // P  # 4 edge sub-blocks of 128 per tile

F32 = mybir.dt.float32
F32R = mybir.dt.float32r
I32 = mybir.dt.int32

# matmul dtype mode: "fp32" (exact, 4 cyc/row) or "fp32r" (1 cyc/row @ N>=512)
MM_MODE = os.environ.get("KERNEL_MM_MODE", "fp32r")


def _mm_ap(ap):
    if MM_MODE == "fp32r":
        return ap.bitcast(F32R)
    return ap


def build_program(e_core: int = E_CORE, num_devices: int = N_CORES):
    """Build and compile the per-core Bass program (SPMD: same program on
    every core, different input slices)."""
    assert e_core % TILE_E == 0
    n_tiles = e_core // TILE_E

    nc = bacc.Bacc(
        "TRN2", target_bir_lowering=False, debug=False, num_devices=num_devices
    )

    src_d = nc.dram_tensor("src", [e_core, NODE_IN], F32, kind="ExternalInput").ap()
    dest_d = nc.dram_tensor("dest", [e_core, NODE_IN], F32, kind="ExternalInput").ap()
    ea_d = nc.dram_tensor("ea", [e_core, EDGE_IN], F32, kind="ExternalInput").ap()
    batch_d = nc.dram_tensor("batch", [e_core], F32, kind="ExternalInput").ap()
    # W1 rearranged on host to [128, 3, 256]:
    #   [:,0,:] = W1[0:128]   (src rows)
    #   [:,1,:] = W1[128:256] (dest rows)
    #   [0:64,2,:] = W1[256:320] (edge_attr rows); rows 64:128 of chunk 2 zero
    w1_d = nc.dram_tensor("w1", [P, 3, HIDDEN], F32, kind="ExternalInput").ap()
    # W1[320:384] (the u rows), natural layout [64, 256]
    w1u_d = nc.dram_tensor("w1u", [GLOBAL_IN, HIDDEN], F32, kind="ExternalInput").ap()
    # W2 rearranged on host to [128, 2, 64]
    w2_d = nc.dram_tensor("w2", [P, 2, EDGE_OUT], F32, kind="ExternalInput").ap()
    b1_d = nc.dram_tensor("b1", [P, 2], F32, kind="ExternalInput").ap()
    b2_d = nc.dram_tensor("b2", [EDGE_OUT, 1], F32, kind="ExternalInput").ap()
    u_d = nc.dram_tensor("u", [B_GLOBAL, GLOBAL_IN], F32, kind="ExternalInput").ap()
    # iota column: rows 64:80 hold 0..15 (partition-aligned with the one-hot
    # rows of the chunk-2 rhs tile), everything else zero
    iota_d = nc.dram_tensor("iota", [P, 1], F32, kind="ExternalInput").ap()
    out_d = nc.dram_tensor("out", [e_core, EDGE_OUT], F32, kind="ExternalOutput").ap()

    with tile.TileContext(nc) as tc, ExitStack() as ctx:
        consts = ctx.enter_context(tc.tile_pool(name="consts", bufs=1))
        loads = ctx.enter_context(tc.tile_pool(name="loads", bufs=3))
        acts = ctx.enter_context(tc.tile_pool(name="acts", bufs=3))
        psum = ctx.enter_context(tc.tile_pool(name="psum", bufs=1, space="PSUM"))

        # ---- setup: constants ------------------------------------------
        ident = consts.tile([P, P], F32)
        make_identity(nc, ident[:])

        w1_sb = consts.tile([P, 3, HIDDEN], F32)
        nc.sync.dma_start(w1_sb[:], w1_d)
        w1u_sb = consts.tile([GLOBAL_IN, HIDDEN], F32)
        nc.sync.dma_start(w1u_sb[:], w1u_d)
        w2_sb = consts.tile([P, 2, EDGE_OUT], F32)
        nc.sync.dma_start(w2_sb[:], w2_d)
        b1_sb = consts.tile([P, 2], F32)
        nc.sync.dma_start(b1_sb[:], b1_d)
        b2_sb = consts.tile([EDGE_OUT, 1], F32)
        nc.sync.dma_start(b2_sb[:], b2_d)
        u_sb = consts.tile([B_GLOBAL, GLOBAL_IN], F32)
        nc.sync.dma_start(u_sb[:], u_d)
        iota_sb = consts.tile([P, 1], F32)
        nc.sync.dma_start(iota_sb[:], iota_d)

        # uW1 = u @ W1u -> [16, 256], landed on partitions 64:80 so it can be
        # copied into w1_sb chunk 2 rows 64:80 without crossing lanes.
        ps_ut = psum.tile([GLOBAL_IN, B_GLOBAL], F32, tag="ps_sT")
        nc.tensor.transpose(ps_ut[:], u_sb[:], ident[:B_GLOBAL, :B_GLOBAL])
        ut_sb = consts.tile([GLOBAL_IN, B_GLOBAL], F32)
        nc.vector.tensor_copy(ut_sb[:], ps_ut[:])
        ps_uw1 = psum.tile([P, HIDDEN], F32, tag="ps_dT")
        nc.tensor.matmul(
            ps_uw1[64:80, :], ut_sb[:], w1u_sb[:], start=True, stop=True
        )
        nc.vector.tensor_copy(w1_sb[64:80, 2, :], ps_uw1[64:80, :])

        # ---- main loop over edge tiles ---------------------------------
        for t in range(n_tiles):
            e0 = t * TILE_E
            esl = slice(e0, e0 + TILE_E)

            # edge-major loads (contiguous in DRAM)
            a_src = loads.tile([P, SUB, NODE_IN], F32, tag="a_src")
            nc.sync.dma_start(a_src[:], src_d[esl].rearrange("(c p) f -> p c f", p=P))
            a_dest = loads.tile([P, SUB, NODE_IN], F32, tag="a_dest")
            nc.sync.dma_start(a_dest[:], dest_d[esl].rearrange("(c p) f -> p c f", p=P))
            a_ea = loads.tile([P, SUB, EDGE_IN], F32, tag="a_ea")
            nc.sync.dma_start(a_ea[:], ea_d[esl].rearrange("(c p) f -> p c f", p=P))

            # chunk-2 rhs tile: rows 0:64 = edge_attr^T, rows 64:80 = one_hot
            chunk2 = acts.tile([80, TILE_E], F32, tag="chunk2")
            nc.sync.dma_start(
                chunk2[64:80, :],
                batch_d[esl][None, :].to_broadcast([B_GLOBAL, TILE_E]),
            )
            nc.gpsimd.tensor_scalar(
                chunk2[64:80, :],
                chunk2[64:80, :],
                iota_sb[64:80, :],
                None,
                mybir.AluOpType.is_equal,
            )

            # transposes: feature-major activations via PE
            ps_sT = psum.tile([P, TILE_E], F32, tag="ps_sT")
            ps_dT = psum.tile([P, TILE_E], F32, tag="ps_dT")
            ps_eT = psum.tile([EDGE_IN, TILE_E], F32, tag="ps_eT")
            for c in range(SUB):
                csl = slice(c * P, (c + 1) * P)
                nc.tensor.transpose(ps_sT[:, csl], a_src[:, c, :], ident[:])
                nc.tensor.transpose(ps_dT[:, csl], a_dest[:, c, :], ident[:])
                nc.tensor.transpose(ps_eT[:, csl], a_ea[:, c, :], ident[:])
            xs = acts.tile([P, TILE_E], F32, tag="xs")
            nc.vector.tensor_copy(xs[:], ps_sT[:])
            xd = acts.tile([P, TILE_E], F32, tag="xd")
            nc.scalar.copy(xd[:], ps_dT[:])
            nc.vector.tensor_copy(chunk2[0:64, :], ps_eT[:])

            # layer 1: h^T = W1^T @ x^T (+b1, relu)  -> [256, 512] as 2 banks
            ps_h0 = psum.tile([P, TILE_E], F32, tag="ps_h0")
            ps_h1 = psum.tile([P, TILE_E], F32, tag="ps_h1")
            for m, ps_h in enumerate((ps_h0, ps_h1)):
                msl = slice(m * P, (m + 1) * P)
                nc.tensor.matmul(
                    ps_h[:], _mm_ap(w1_sb[:, 0, msl]), _mm_ap(xs[:]),
                    start=True, stop=False,
                )
                nc.tensor.matmul(
                    ps_h[:], _mm_ap(w1_sb[:, 1, msl]), _mm_ap(xd[:]),
                    start=False, stop=False,
                )
                nc.tensor.matmul(
                    ps_h[:], _mm_ap(w1_sb[0:80, 2, msl]), _mm_ap(chunk2[:]),
                    start=False, stop=True,
                )
            h = acts.tile([P, 2, TILE_E], F32, tag="h")
            nc.scalar.activation(
                h[:, 0, :], ps_h0[:], mybir.ActivationFunctionType.Relu,
                bias=b1_sb[:, 0:1],
            )
            nc.scalar.activation(
                h[:, 1, :], ps_h1[:], mybir.ActivationFunctionType.Relu,
                bias=b1_sb[:, 1:2],
            )

            # layer 2: out^T = W2^T @ h^T (+b2) -> [64, 512]
            ps_o = psum.tile([EDGE_OUT, TILE_E], F32, tag="ps_o")
            nc.tensor.matmul(
                ps_o[:], _mm_ap(w2_sb[:, 0, :]), _mm_ap(h[:, 0, :]),
                start=True, stop=False,
            )
            nc.tensor.matmul(
                ps_o[:], _mm_ap(w2_sb[:, 1, :]), _mm_ap(h[:, 1, :]),
                start=False, stop=True,
            )
            o_sb = acts.tile([EDGE_OUT, TILE_E], F32, tag="o_sb")
            nc.scalar.activation(
                o_sb[:], ps_o[:], mybir.ActivationFunctionType.Identity,
                bias=b2_sb[:],
            )

            # transpose back to edge-major and store contiguously
            ps_oT = psum.tile([P, SUB * EDGE_OUT], F32, tag="ps_oT")
            for c in range(SUB):
                nc.tensor.transpose(
                    ps_oT[:, c * EDGE_OUT : (c + 1) * EDGE_OUT],
                    o_sb[:, c * P : (c + 1) * P],
                    ident[:EDGE_OUT, :EDGE_OUT],
                )
            oT = acts.tile([P, SUB, EDGE_OUT], F32, tag="oT")
            nc.vector.tensor_copy(oT[:], ps_oT[:])
            nc.sync.dma_start(out_d[esl].rearrange("(c p) f -> p c f", p=P), oT[:])

    nc.compile()
    return nc


def make_in_maps(inputs: dict, e_core: int = E_CORE, n_cores: int = N_CORES):
    src = np.ascontiguousarray(np.asarray(inputs["src"], dtype=np.float32))
    dest = np.ascontiguousarray(np.asarray(inputs["dest"], dtype=np.float32))
    ea = np.ascontiguousarray(np.asarray(inputs["edge_attr"], dtype=np.float32))
    u = np.ascontiguousarray(np.asarray(inputs["u"], dtype=np.float32))
    batch = np.ascontiguousarray(np.asarray(inputs["batch"]).astype(np.int32))
    W1 = np.asarray(inputs["W1"], dtype=np.float32)
    b1 = np.asarray(inputs["b1"], dtype=np.float32)
    W2 = np.asarray(inputs["W2"], dtype=np.float32)
    b2 = np.asarray(inputs["b2"], dtype=np.float32)

    # host-side weight layout shuffles (no arithmetic)
    w1_r = np.zeros((P, 3, HIDDEN), dtype=np.float32)
    w1_r[:, 0, :] = W1[0:128]
    w1_r[:, 1, :] = W1[128:256]
    w1_r[0:64, 2, :] = W1[256:320]
    w1u = np.ascontiguousarray(W1[320:384])
    w2_r = np.ascontiguousarray(
        W2.reshape(2, P, EDGE_OUT).transpose(1, 0, 2)
    )
    b1_r = np.ascontiguousarray(b1.reshape(2, P).T)
    b2_r = np.ascontiguousarray(b2.reshape(EDGE_OUT, 1))
    iota = np.zeros((P, 1), dtype=np.float32)
    iota[64:80, 0] = np.arange(16)

    in_maps = []
    for c in range(n_cores):
        esl = slice(c * e_core, (c + 1) * e_core)
        in_maps.append(
            {
                "src": src[esl],
                "dest": dest[esl],
                "ea": ea[esl],
                "batch": batch[esl].astype(np.float32),
                "w1": w1_r,
                "w1u": w1u,
                "w2": w2_r,
                "b1": b1_r,
                "b2": b2_r,
                "u": u,
                "iota": iota,
            }
        )
    return in_maps


_CACHED_NC = None


def kernel(**inputs) -> np.ndarray:
    global _CACHED_NC
    if _CACHED_NC is None:
        _CACHED_NC = build_program()
    nc = _CACHED_NC
    in_maps = make_in_maps(inputs)
    res = run_bass_kernel_spmd(nc, in_maps, core_ids=list(range(N_CORES)))
    out = np.concatenate([res.results[c]["out"] for c in range(N_CORES)], axis=0)
    return out


# revision 6
# speedup vs baseline: 2.0143x; 2.0143x over previous
"""Trainium2 Bass kernel for the EdgeModel GNN message-passing MLP.

Computation (per edge e):
    x = concat([src[e], dest[e], edge_attr[e], u[batch[e]]])   # [384]
    h = relu(x @ W1 + b1)                                      # [256]
    out[e] = h @ W2 + b2                                       # [64]

Sharding: data-parallel over the edge dimension E across 8 NeuronCores;
u and the MLP weights are replicated. No cross-device communication.

Device algorithm (per core, E_core = 65536 edges, tiles of 512 edges):
  - The TensorE contraction dim must live on partitions, so the x operand
    must be feature-major.  We keep W1/W2 as the stationary operand in
    their natural (feature-major) layout and transpose the activations:
      * src/dest/edge_attr tiles are loaded edge-major (contiguous DMA)
        and transposed on the PE (matmul-with-identity transpose).
      * u[batch] is folded into W1: the last contraction chunk is
        [W1_ea (64 rows); u @ W1_u (16 rows)] against a rhs of
        [edge_attr^T (64); one_hot(batch) (16)].  one_hot is built with a
        DMA-replicated batch row compared against an iota column.
      * Layer 1 emits h^T (hidden-major), which is exactly the layout
        layer 2 needs; only the final [64, e] output tile needs a
        transpose back to edge-major before the contiguous store.
"""

import os
import sys

for _p in ("/opt/trn_rl_repo", os.path.expanduser("~/.axon_site/_ro/trn_rl_repo")):
    if os.path.isdir(_p) and _p not in sys.path:
        sys.path.insert(0, _p)

from contextlib import ExitStack

import numpy as np

import concourse.bacc as bacc
import concourse.bass as bass
import concourse.mybir as mybir
import concourse.tile as tile
from concourse.bass_utils import run_bass_kernel_spmd
from concourse.masks import make_identity

N_CORES = 8
E_FULL = 524288
E_CORE = E_FULL // N_CORES
NODE_IN = 128
EDGE_IN = 64
GLOBAL_IN = 64
B_GLOBAL = 16
HIDDEN = 256
EDGE_OUT = 64
P = 128
TILE_E = 512
SUB = TILE_E // P  # 4 edge sub-blocks of 128 per tile

F32 = mybir.dt.float32
F32R = mybir.dt.float32r
I32 = mybir.dt.int32

# matmul dtype mode: "fp32" (exact, 4 cyc/row) or "fp32r" (~1.5e-4 rel err,
# 1 cyc/row at N>=256).  f32r operands must be produced (rounded) by a
# compute op, so all matmul operand tiles use MMDT and are written by
# DVE/ACT/GPSIMD ops, never directly by DMA.
MM_MODE = os.environ.get("KERNEL_MM_MODE", "fp32r")
MMDT = F32R if MM_MODE == "fp32r" else F32


def build_program(e_core: int = E_CORE, num_devices: int = N_CORES):
    """Build and compile the per-core Bass program (SPMD: same program on
    every core, different input slices)."""
    assert e_core % TILE_E == 0
    n_tiles = e_core // TILE_E

    nc = bacc.Bacc(
        "TRN2", target_bir_lowering=False, debug=False, num_devices=num_devices
    )

    src_d = nc.dram_tensor("src", [e_core, NODE_IN], F32, kind="ExternalInput").ap()
    dest_d = nc.dram_tensor("dest", [e_core, NODE_IN], F32, kind="ExternalInput").ap()
    ea_d = nc.dram_tensor("ea", [e_core, EDGE_IN], F32, kind="ExternalInput").ap()
    batch_d = nc.dram_tensor("batch", [e_core], F32, kind="ExternalInput").ap()
    # W1 rearranged on host to [128, 3, 256]:
    #   [:,0,:] = W1[0:128]   (src rows)
    #   [:,1,:] = W1[128:256] (dest rows)
    #   [0:64,2,:] = W1[256:320] (edge_attr rows); rows 64:128 of chunk 2 zero
    w1_d = nc.dram_tensor("w1", [P, 3, HIDDEN], F32, kind="ExternalInput").ap()
    # W1[320:384] (the u rows), natural layout [64, 256]
    w1u_d = nc.dram_tensor("w1u", [GLOBAL_IN, HIDDEN], F32, kind="ExternalInput").ap()
    # W2 rearranged on host to [128, 2, 64]
    w2_d = nc.dram_tensor("w2", [P, 2, P], F32, kind="ExternalInput").ap()
    b1_d = nc.dram_tensor("b1", [P, 2], F32, kind="ExternalInput").ap()
    b2_d = nc.dram_tensor("b2", [EDGE_OUT, 1], F32, kind="ExternalInput").ap()
    u_d = nc.dram_tensor("u", [B_GLOBAL, GLOBAL_IN], F32, kind="ExternalInput").ap()
    # iota column: rows 64:80 hold 0..15 (partition-aligned with the one-hot
    # rows of the chunk-2 rhs tile), everything else zero
    iota_d = nc.dram_tensor("iota", [P, 1], F32, kind="ExternalInput").ap()
    out_d = nc.dram_tensor("out", [e_core, EDGE_OUT], F32, kind="ExternalOutput").ap()

    with tile.TileContext(nc) as tc, ExitStack() as ctx:
        consts = ctx.enter_context(tc.tile_pool(name="consts", bufs=1))
        loads = ctx.enter_context(tc.tile_pool(name="loads", bufs=3))
        acts = ctx.enter_context(tc.tile_pool(name="acts", bufs=3))
        psum = ctx.enter_context(tc.tile_pool(name="psum", bufs=1, space="PSUM"))

        # ---- setup: constants ------------------------------------------
        ident = consts.tile([P, P], F32)
        make_identity(nc, ident[:])

        w1_ld = consts.tile([P, 3, HIDDEN], F32)
        nc.sync.dma_start(w1_ld[:], w1_d)
        w1_sb = consts.tile([P, 3, HIDDEN], MMDT)
        nc.vector.tensor_copy(w1_sb[:], w1_ld[:])
        w1u_sb = consts.tile([GLOBAL_IN, HIDDEN], F32)
        nc.sync.dma_start(w1u_sb[:], w1u_d)
        w2_ld = consts.tile([P, 2, P], F32)
        nc.sync.dma_start(w2_ld[:], w2_d)
        w2_sb = consts.tile([P, 2, P], MMDT)
        nc.vector.tensor_copy(w2_sb[:], w2_ld[:])
        b1_sb = consts.tile([P, 2], F32)
        nc.sync.dma_start(b1_sb[:], b1_d)
        b2_sb = consts.tile([EDGE_OUT, 1], F32)
        nc.sync.dma_start(b2_sb[:], b2_d)
        u_sb = consts.tile([B_GLOBAL, GLOBAL_IN], F32)
        nc.sync.dma_start(u_sb[:], u_d)
        iota_sb = consts.tile([P, 1], F32)
        nc.sync.dma_start(iota_sb[:], iota_d)

        # uW1 = u @ W1u -> [16, 256], landed on partitions 64:80 so it can be
        # copied into w1_sb chunk 2 rows 64:80 without crossing lanes.
        ps_ut = psum.tile([GLOBAL_IN, B_GLOBAL], F32, tag="ps_sT")
        nc.tensor.transpose(ps_ut[:], u_sb[:], ident[:B_GLOBAL, :B_GLOBAL])
        ut_sb = consts.tile([GLOBAL_IN, B_GLOBAL], F32)
        nc.vector.tensor_copy(ut_sb[:], ps_ut[:])
        ps_uw1 = psum.tile([P, HIDDEN], F32, tag="ps_dT")
        nc.tensor.matmul(
            ps_uw1[64:80, :], ut_sb[:], w1u_sb[:], start=True, stop=True
        )
        nc.vector.tensor_copy(w1_sb[64:80, 2, :], ps_uw1[64:80, :])

        # ---- main loop over edge tiles ---------------------------------
        for t in range(n_tiles):
            e0 = t * TILE_E
            esl = slice(e0, e0 + TILE_E)

            # edge-major loads (contiguous in DRAM)
            a_src = loads.tile([P, SUB, NODE_IN], F32, tag="a_src")
            nc.sync.dma_start(a_src[:], src_d[esl].rearrange("(c p) f -> p c f", p=P))
            a_dest = loads.tile([P, SUB, NODE_IN], F32, tag="a_dest")
            nc.sync.dma_start(a_dest[:], dest_d[esl].rearrange("(c p) f -> p c f", p=P))
            a_ea = loads.tile([P, SUB, EDGE_IN], F32, tag="a_ea")
            nc.sync.dma_start(a_ea[:], ea_d[esl].rearrange("(c p) f -> p c f", p=P))

            # chunk-2 rhs tile: rows 0:64 = edge_attr^T, rows 64:80 = one_hot
            chunk2 = acts.tile([80, TILE_E], MMDT, tag="chunk2")
            b_bcast = loads.tile([80, TILE_E], F32, tag="b_bcast")
            nc.sync.dma_start(
                b_bcast[64:80, :],
                batch_d[esl][None, :].to_broadcast([B_GLOBAL, TILE_E]),
            )
            nc.vector.tensor_scalar(
                chunk2[64:80, :],
                b_bcast[64:80, :],
                iota_sb[64:80, :],
                None,
                mybir.AluOpType.is_equal,
            )

            # transposes: feature-major activations via PE
            ps_sT = psum.tile([P, TILE_E], F32, tag="ps_sT")
            ps_dT = psum.tile([P, TILE_E], F32, tag="ps_dT")
            ps_eT = psum.tile([EDGE_IN, TILE_E], F32, tag="ps_eT")
            for c in range(SUB):
                csl = slice(c * P, (c + 1) * P)
                nc.tensor.transpose(ps_sT[:, csl], a_src[:, c, :], ident[:])
                nc.tensor.transpose(ps_dT[:, csl], a_dest[:, c, :], ident[:])
                nc.tensor.transpose(ps_eT[:, csl], a_ea[:, c, :], ident[:])
            xs = acts.tile([P, TILE_E], MMDT, tag="xs")
            nc.vector.tensor_copy(xs[:], ps_sT[:])
            xd = acts.tile([P, TILE_E], MMDT, tag="xd")
            nc.scalar.copy(xd[:], ps_dT[:])
            nc.vector.tensor_copy(chunk2[0:64, :], ps_eT[:])

            # layer 1: h^T = W1^T @ x^T (+b1, relu)  -> [256, 512] as 2 banks
            ps_h0 = psum.tile([P, TILE_E], F32, tag="ps_h0")
            ps_h1 = psum.tile([P, TILE_E], F32, tag="ps_h1")
            for m, ps_h in enumerate((ps_h0, ps_h1)):
                msl = slice(m * P, (m + 1) * P)
                nc.tensor.matmul(
                    ps_h[:], w1_sb[:, 0, msl], xs[:],
                    start=True, stop=False,
                )
                nc.tensor.matmul(
                    ps_h[:], w1_sb[:, 1, msl], xd[:],
                    start=False, stop=False,
                )
                nc.tensor.matmul(
                    ps_h[:], w1_sb[0:80, 2, msl], chunk2[:],
                    start=False, stop=True,
                )
            h = acts.tile([P, 2, TILE_E], MMDT, tag="h")
            nc.scalar.activation(
                h[:, 0, :], ps_h0[:], mybir.ActivationFunctionType.Relu,
                bias=b1_sb[:, 0:1],
            )
            nc.scalar.activation(
                h[:, 1, :], ps_h1[:], mybir.ActivationFunctionType.Relu,
                bias=b1_sb[:, 1:2],
            )

            # layer 2: out^T = W2^T @ h^T (+b2) -> [64, 512]
            ps_o = psum.tile([P, TILE_E], F32, tag="ps_o")
            nc.tensor.matmul(
                ps_o[:], w2_sb[:, 0, :], h[:, 0, :],
                start=True, stop=False,
            )
            nc.tensor.matmul(
                ps_o[:], w2_sb[:, 1, :], h[:, 1, :],
                start=False, stop=True,
            )
            o_sb = acts.tile([EDGE_OUT, TILE_E], F32, tag="o_sb")
            nc.scalar.activation(
                o_sb[:], ps_o[0:EDGE_OUT, :], mybir.ActivationFunctionType.Identity,
                bias=b2_sb[:],
            )

            # transpose back to edge-major and store contiguously
            ps_oT = psum.tile([P, SUB * EDGE_OUT], F32, tag="ps_oT")
            for c in range(SUB):
                nc.tensor.transpose(
                    ps_oT[:, c * EDGE_OUT : (c + 1) * EDGE_OUT],
                    o_sb[:, c * P : (c + 1) * P],
                    ident[:EDGE_OUT, :EDGE_OUT],
                )
            oT = acts.tile([P, SUB, EDGE_OUT], F32, tag="oT")
            nc.vector.tensor_copy(oT[:], ps_oT[:])
            nc.sync.dma_start(out_d[esl].rearrange("(c p) f -> p c f", p=P), oT[:])

    nc.compile()
    return nc


def make_in_maps(inputs: dict, e_core: int = E_CORE, n_cores: int = N_CORES):
    src = np.ascontiguousarray(np.asarray(inputs["src"], dtype=np.float32))
    dest = np.ascontiguousarray(np.asarray(inputs["dest"], dtype=np.float32))
    ea = np.ascontiguousarray(np.asarray(inputs["edge_attr"], dtype=np.float32))
    u = np.ascontiguousarray(np.asarray(inputs["u"], dtype=np.float32))
    batch = np.ascontiguousarray(np.asarray(inputs["batch"]).astype(np.int32))
    W1 = np.asarray(inputs["W1"], dtype=np.float32)
    b1 = np.asarray(inputs["b1"], dtype=np.float32)
    W2 = np.asarray(inputs["W2"], dtype=np.float32)
    b2 = np.asarray(inputs["b2"], dtype=np.float32)

    # host-side weight layout shuffles (no arithmetic)
    w1_r = np.zeros((P, 3, HIDDEN), dtype=np.float32)
    w1_r[:, 0, :] = W1[0:128]
    w1_r[:, 1, :] = W1[128:256]
    w1_r[0:64, 2, :] = W1[256:320]
    w1u = np.ascontiguousarray(W1[320:384])
    w2_r = np.zeros((P, 2, P), dtype=np.float32)
    w2_r[:, :, :EDGE_OUT] = W2.reshape(2, P, EDGE_OUT).transpose(1, 0, 2)
    b1_r = np.ascontiguousarray(b1.reshape(2, P).T)
    b2_r = np.ascontiguousarray(b2.reshape(EDGE_OUT, 1))
    iota = np.zeros((P, 1), dtype=np.float32)
    iota[64:80, 0] = np.arange(16)

    in_maps = []
    for c in range(n_cores):
        esl = slice(c * e_core, (c + 1) * e_core)
        in_maps.append(
            {
                "src": src[esl],
                "dest": dest[esl],
                "ea": ea[esl],
                "batch": batch[esl].astype(np.float32),
                "w1": w1_r,
                "w1u": w1u,
                "w2": w2_r,
                "b1": b1_r,
                "b2": b2_r,
                "u": u,
                "iota": iota,
            }
        )
    return in_maps


_CACHED_NC = None


def kernel(**inputs) -> np.ndarray:
    global _CACHED_NC
    if _CACHED_NC is None:
        _CACHED_NC = build_program()
    nc = _CACHED_NC
    in_maps = make_in_maps(inputs)
    res = run_bass_kernel_spmd(nc, in_maps, core_ids=list(range(N_CORES)))
    out = np.concatenate([res.results[c]["out"] for c in range(N_CORES)], axis=0)
    return out
